# revision 22
# baseline (speedup 1.0000x reference)
"""MLA (multi-latent attention) prefill kernel for Trainium2, 8 NeuronCores.

Tensor-parallel over heads: each of the 8 cores owns 2 of the 16 heads.
w_q / w_kv_b are column-sharded, w_o row-sharded; the small kv_a latent
projection is replicated. Per-core partial outputs are summed on the host
(the "all-reduce" of the o_proj).

v3 dataflow (per core, column-major [feature, seq] layouts):
  qT   = wq_mod.T  @ hT     [256, S]  (2 M-blocks: h0[nope|x'], h1[nope|x'])
  kvaT = wkva_mod.T @ hT    [640, S]  (latent 512, then k-x' twice --
  the duplicate keeps the stationary 128 wide, narrow matmuls measure ~35%
  slower per row)
  RoPE: x' = interleaved pe cols (folded into weights); the rotate-half
  operand y is built as signed partition-shifted DVE multiplies against
  partition-aligned +-sin tables; adds run on GpSimd.
  rmsnorm: latent staged to SBUF in bf16 (stg); squares on GpSimd; the
  partition+block reduction is a ones-matmul on the PE; rsqrt = quake
  bit-trick seed + 2 Newton steps on [128,4]-transposed columns (DVE ops
  ~170ns each, no ACT tables touched).  kv_b consumes the UNNORMALIZED
  bf16 latent, and the norm scale folds into the kT writes (broadcast row
  multiply) and v writes (per-partition tensor_scalar) -- so no PE matmul
  ever waits on the rsqrt chain.
  Attention is computed TRANSPOSED: scoresT[k, q] = kT_blk^T @ qT so the
  exp (ACT) writes probsT straight to SBUF -- no PE transposes and no
  PSUM->SBUF prob copies. The ACT engine only ever runs Exp and Copy
  (one activation-table load for the whole kernel).  Causal masking =
  gpsimd.affine_select zeroing invalid probs on diagonal key-blocks.
  Softmax sums come from a ones matmul accumulated in PSUM; 1/sum is
  computed on [128,4]-transposed columns and folded into the PSUM->SBUF
  copy of attnT.  o = attnT.T @ w_o chunks, both heads accumulated in
  PSUM, streamed out.

Matmul operands are bf16 (full-rate PE, fp32 PSUM accumulation); softmax
statistics and rmsnorm statistics stay fp32.
"""
import os
import sys
import types
import numpy as np
import ml_dtypes

import concourse.bass as bass
import concourse.mybir as mybir
import concourse.tile as tile
from concourse import bacc, bass_isa, bass_utils, masks

F32 = mybir.dt.float32
BF16 = mybir.dt.bfloat16
I32 = mybir.dt.int32

S, HID = 2048, 2048
H, NOPE, ROPE, VD, KLR = 16, 64, 64, 128, 512
QD = NOPE + ROPE          # 128
SCALE = QD ** -0.5
EPS = 1e-6
NCORES = 8
HPC = H // NCORES         # heads per core = 2

SC = 512                  # seq chunk for projections
NSC = S // SC             # 4
HC = HID // 128           # 16 hid chunks
EXPB = 20.0               # fixed exp bias (overflow headroom)


def build_nc():
    nc = bacc.Bacc("TRN2", target_bir_lowering=False, debug=False,
                   num_devices=NCORES)
    dr = {}
    dr["hT"] = nc.dram_tensor("hT", [HID, S], BF16, kind="ExternalInput")
    dr["wq"] = nc.dram_tensor("wq", [HID, 256], BF16, kind="ExternalInput")
    dr["wkva"] = nc.dram_tensor("wkva", [HID, 640], BF16, kind="ExternalInput")
    dr["wkvb"] = nc.dram_tensor("wkvb", [KLR, 384], BF16, kind="ExternalInput")
    dr["wo"] = nc.dram_tensor("wo", [HPC * VD, HID], BF16, kind="ExternalInput")
    dr["cosd"] = nc.dram_tensor("cosd", [128, S], BF16, kind="ExternalInput")
    dr["msind"] = nc.dram_tensor("msind", [128, S], BF16, kind="ExternalInput")
    dr["o"] = nc.dram_tensor("o", [S, HID], BF16, kind="ExternalOutput")

    with tile.TileContext(nc) as tc:
        build_tile_kernel(nc, tc, {k: v.ap() for k, v in dr.items()})
    nc.compile()
    return nc


def build_tile_kernel(nc, tc, d):
    from contextlib import ExitStack
    with ExitStack() as ctx:
        _build_tile_kernel(nc, tc, d, ctx)


def _build_tile_kernel(nc, tc, d, ctx):
    AF = mybir.ActivationFunctionType
    ALU = mybir.AluOpType

    consts = ctx.enter_context(tc.tile_pool(name="consts", bufs=1))
    big = ctx.enter_context(tc.tile_pool(name="big", bufs=1))
    work = ctx.enter_context(tc.tile_pool(name="work", bufs=2))
    stat = ctx.enter_context(tc.tile_pool(name="stat", bufs=2))
    outp = ctx.enter_context(tc.tile_pool(name="outp", bufs=2))
    ps = ctx.enter_context(tc.tile_pool(name="ps", bufs=8, space="PSUM"))

    # ---- input DMAs ------------------------------------------------------
    # full hT resident in SBUF; chunk-0 pieces land first
    hT_sb = consts.tile([128, HC, S], BF16)
    cos_sb = consts.tile([128, S], BF16)
    msin_sb = consts.tile([128, S], BF16)
    wkvb_sb = consts.tile([128, 4, 384], BF16)
    wo_sb = consts.tile([128, HPC, HID], BF16)
    for c in range(NSC):
        cs = slice(c * SC, (c + 1) * SC)
        for kp in range(HC // 2):
            nc.sync.dma_start(
                out=hT_sb[:, 2 * kp:2 * kp + 2, cs],
                in_=d["hT"][256 * kp:256 * (kp + 1), cs].rearrange(
                    "(k p) m -> p k m", p=128))
        if c == 0:
            nc.sync.dma_start(out=cos_sb[:], in_=d["cosd"])
            nc.sync.dma_start(out=msin_sb[:], in_=d["msind"])
            nc.sync.dma_start(out=wkvb_sb[:],
                              in_=d["wkvb"].rearrange("(k p) m -> p k m", p=128))
    wq_sb = consts.tile([128, HC, 256], BF16)
    wkva_sb = consts.tile([128, HC, 640], BF16)
    wq_r = d["wq"].rearrange("(k p) m -> p k m", p=128)
    wkva_r = d["wkva"].rearrange("(k p) m -> p k m", p=128)
    for k in range(HC):
        nc.scalar.dma_start(out=wq_sb[:, k, :], in_=wq_r[:, k, :])
        nc.scalar.dma_start(out=wkva_sb[:, k, :], in_=wkva_r[:, k, :])
    nc.sync.dma_start(out=wo_sb[:], in_=d["wo"].rearrange("(h p) n -> p h n", p=128))

    ones_bf = consts.tile([128, 128], BF16)
    nc.vector.memset(ones_bf[:], 1.0)
    one_f32 = consts.tile([1, 1], F32)
    nc.vector.memset(one_f32[:], 1.0)
    ident = consts.tile([128, 128], BF16)
    masks.make_identity(nc, ident[:])
    nexpb_sb = consts.tile([128, 1], F32)
    nc.vector.memset(nexpb_sb[:], -EXPB)
    ones_row = consts.tile([1, 128], BF16)
    nc.vector.memset(ones_row[:], 1.0)

    # ---- persistent activations -----------------------------------------
    qT = [big.tile([128, S], BF16, tag=f"qT{h}", name=f"qT{h}") for h in range(HPC)]
    kT = [big.tile([128, S], BF16, tag=f"kT{h}", name=f"kT{h}") for h in range(HPC)]
    v_sb = big.tile([128, S // 128, HPC * VD], BF16, tag="v")

    # =====================================================================
    def proj_w1(c):
        """q + shared k_pe wave and the rope epilogue."""
        cs = slice(c * SC, (c + 1) * SC)
        pq = [ps.tile([128, SC], F32, tag="ps", name=f"pq{i}") for i in range(HPC)]
        pkpe = ps.tile([128, SC], F32, tag="ps", name="pkpe")
        for k in range(HC):
            for h in range(HPC):
                nc.tensor.matmul(pq[h][:], wq_sb[:, k, h * 128:(h + 1) * 128],
                                 hT_sb[:, k, cs], start=(k == 0),
                                 stop=(k == HC - 1))
            nc.tensor.matmul(pkpe[:], wkva_sb[:, k, 512:640],
                             hT_sb[:, k, cs], start=(k == 0),
                             stop=(k == HC - 1))
        # rope: q' = x'*cos + y*sin, y = signed rotate-half of x'
        for h in range(HPC):
            nc.vector.tensor_copy(qT[h][0:64, cs], pq[h][0:64, :])
            t2 = work.tile([128, SC], F32, tag="t2", bufs=4)
            t3 = work.tile([128, SC], F32, tag="t2", bufs=4)
            nc.vector.tensor_tensor(t2[64:96, :], pq[h][96:128, :],
                                    msin_sb[96:128, cs], ALU.mult)
            nc.vector.tensor_tensor(t2[96:128, :], pq[h][64:96, :],
                                    msin_sb[64:96, cs], ALU.mult)
            nc.vector.tensor_tensor(t3[64:128, :], pq[h][64:128, :],
                                    cos_sb[64:128, cs], ALU.mult)
            nc.vector.tensor_tensor(qT[h][64:128, cs], t3[64:128, :],
                                    t2[64:128, :], ALU.add)
        tk = work.tile([128, SC], F32, tag="t2", bufs=4)
        tk3 = work.tile([128, SC], F32, tag="t2", bufs=4)
        nc.vector.tensor_tensor(tk[64:96, :], pkpe[32:64, :],
                                msin_sb[32:64, cs], ALU.mult)
        nc.vector.tensor_tensor(tk[96:128, :], pkpe[0:32, :],
                                msin_sb[0:32, cs], ALU.mult)
        nc.vector.tensor_tensor(tk3[64:128, :], pkpe[0:64, :],
                                cos_sb[0:64, cs], ALU.mult)
        nc.vector.tensor_tensor(kT[0][64:128, cs], tk3[64:128, :],
                                tk[64:128, :], ALU.add)
        nc.vector.tensor_copy(kT[1][64:128, cs], kT[0][64:128, cs])

    def proj_w2(c):
        """latent wave; stage to bf16 SBUF; squares on GpSimd."""
        cs = slice(c * SC, (c + 1) * SC)
        plat = [ps.tile([128, SC], F32, tag="ps", name=f"plat{i}") for i in range(4)]
        for k in range(HC):
            for m in range(4):
                nc.tensor.matmul(plat[m][:], wkva_sb[:, k, m * 128:(m + 1) * 128],
                                 hT_sb[:, k, cs], start=(k == 0),
                                 stop=(k == HC - 1))
        stg = work.tile([128, 4, SC], BF16, tag="stg", bufs=2)
        nc.vector.tensor_copy(stg[:, 0, :], plat[0][:])
        nc.vector.tensor_copy(stg[:, 1, :], plat[1][:])
        nc.scalar.copy(stg[:, 2, :], plat[2][:])
        nc.scalar.copy(stg[:, 3, :], plat[3][:])
        sq = work.tile([128, 4, SC], BF16, tag="sq", bufs=1)
        nc.scalar.activation(sq[:], stg[:], AF.Square)
        return stg, sq

    # rmsnorm scale chain, split so the PE pieces slot between waves and
    # no PE matmul ever waits on it (kv_b reads the unnormalized latent).
    def stats_a(sq):
        """PE reduction of sum-of-squares + row copy to SBUF."""
        pssq = ps.tile([128, SC], F32, tag="ps", name="pssq")
        for m in range(4):
            nc.tensor.matmul(pssq[:], ones_bf[:], sq[:, m, :],
                             start=(m == 0), stop=(m == 3))
        srow = stat.tile([1, SC], F32, tag="srow", name="srow")
        nc.vector.tensor_copy(srow[:], pssq[0:1, :])
        return srow

    def stats_b1(srow):
        """transpose sums to [128,4] columns; quake rsqrt on the columns."""
        pcol = ps.tile([128, 4], F32, tag="ps", name="pcolq")
        for qi in range(4):
            nc.tensor.transpose(pcol[:, qi:qi + 1],
                                srow[0:1, qi * 128:(qi + 1) * 128],
                                one_f32[0:1, 0:1])
        mt = stat.tile([128, 4], F32, tag="mt", name="mt")
        nc.vector.tensor_scalar(out=mt[:], in0=pcol[:], scalar1=1.0 / KLR,
                                scalar2=EPS, op0=ALU.mult, op1=ALU.add)
        ti = stat.tile([128, 4], I32, tag="ti", name="ti")
        nc.vector.tensor_scalar(out=ti[:], in0=mt.bitcast(I32)[:],
                                scalar1=1, scalar2=None,
                                op0=ALU.logical_shift_right)
        yt = stat.tile([128, 4], F32, tag="yt", name="yt")
        nc.vector.tensor_scalar(out=yt.bitcast(I32)[:], in0=ti[:],
                                scalar1=-1, scalar2=0x5F3759DF,
                                op0=ALU.mult, op1=ALU.add)
        y2 = stat.tile([128, 4], F32, tag="y2", name="y2")
        for _ in range(2):
            nc.vector.tensor_tensor(y2[:], yt[:], yt[:], ALU.mult)
            nc.vector.scalar_tensor_tensor(out=y2[:], in0=y2[:], scalar=-0.5,
                                           in1=mt[:], op0=ALU.mult,
                                           op1=ALU.mult)
            nc.vector.scalar_tensor_tensor(out=yt[:], in0=y2[:], scalar=1.5,
                                           in1=yt[:], op0=ALU.add,
                                           op1=ALU.mult)
        yb = stat.tile([128, 4], BF16, tag="yb", name="yb")
        nc.vector.tensor_copy(yb[:], yt[:])
        return yt, yb

    def stats_b2(yb):
        """columns back to a row; broadcast to all partitions."""
        prt = ps.tile([1, SC], BF16, tag="ps", name="prtq")
        for qi in range(4):
            nc.tensor.transpose(prt[0:1, qi * 128:(qi + 1) * 128],
                                yb[:, qi:qi + 1], ident[:])
        rrbf = stat.tile([1, SC], BF16, tag="rrbfq", name="rrbfq")
        nc.vector.tensor_copy(rrbf[:], prt[0:1, :])
        sbcp = ps.tile([128, SC], F32, tag="ps", name="sbcp")
        nc.tensor.matmul(sbcp[:], ones_row[0:1, :], rrbf[0:1, :],
                         start=True, stop=True)
        sbcb = work.tile([128, SC], BF16, tag="sbcb", bufs=2)
        nc.scalar.copy(sbcb[:], sbcp[:])
        return sbcb

    def proj_mm2(c, stg, sbcb, yt):
        """kv_b from the unnormalized latent; norm scale folded into the
        kT (row broadcast multiply) and v (per-partition scalar) writes."""
        cs = slice(c * SC, (c + 1) * SC)
        pnope = ps.tile([128, SC], F32, tag="ps", name="pnope")
        for kk in range(4):
            nc.tensor.matmul(pnope[:], wkvb_sb[:, kk, 0:128], stg[:, kk, :],
                             start=(kk == 0), stop=(kk == 3))
        nc.vector.tensor_tensor(kT[0][0:64, cs], pnope[0:64, :],
                                sbcb[0:64, :], ALU.mult)
        nc.vector.tensor_tensor(kT[1][0:64, cs], pnope[64:128, :],
                                sbcb[64:128, :], ALU.mult)
        for t in range(4):
            pv = ps.tile([128, HPC * VD], F32, tag="ps", name="pv")
            for kk in range(4):
                nc.tensor.matmul(pv[:], stg[:, kk, t * 128:(t + 1) * 128],
                                 wkvb_sb[:, kk, 128:384],
                                 start=(kk == 0), stop=(kk == 3))
            nc.vector.tensor_scalar_mul(v_sb[:, 4 * c + t, :], pv[:],
                                        yt[:, t:t + 1])

    # =====================================================================
    def _norm_chain(srow_h, h):
        """serow -> [128,4] columns -> reciprocal -> back to a [1,512] row."""
        pc = ps.tile([128, 4], F32, tag="ps", name=f"pcol{h}")
        for qi in range(4):
            nc.tensor.transpose(pc[:, qi:qi + 1],
                                srow_h[0:1, qi * 128:(qi + 1) * 128],
                                one_f32[0:1, 0:1])
        r4 = stat.tile([128, 4], F32, tag="r4", name="r4")
        nc.vector.reciprocal(r4[:], pc[:])
        rb = stat.tile([128, 4], BF16, tag="rb4", name="rb4")
        nc.vector.tensor_copy(rb[:], r4[:])
        pr = ps.tile([1, 512], BF16, tag="ps", name=f"prt{h}")
        for qi in range(4):
            nc.tensor.transpose(pr[0:1, qi * 128:(qi + 1) * 128],
                                rb[:, qi:qi + 1], ident[:])
        return pc, rb, pr

    def attn_core(B):
        """Transposed-scores attention for superblock B (512 queries), both
        heads.  Per key-block kt: scoresT (PE) -> exp (ACT, writes probsT to
        SBUF) -> [causal zero via affine_select on diagonal blocks (gpsimd)]
        -> attnT accumulate + ones sum accumulate (PE).  The 1/sumexp scale
        is folded into the PSUM->SBUF copy of attnT; its reciprocal chain
        (transpose->recip->transpose->broadcast) is scheduled after both
        heads so the PE never waits on it."""
        nkt = 4 * (B + 1)
        LAG = 2
        pa = [None, None]
        pone = [None, None]
        serow = [None, None]
        at = [None, None]
        for h in range(HPC):
            pa[h] = ps.tile([128, 512], F32, tag="ps", name=f"pa{h}")
            pone[h] = ps.tile([128, 512], F32, tag="ps", name=f"pone{h}")
            pts = {}
            for step in range(nkt + LAG):
                if step < nkt:
                    kt = step
                    qoff = max(0, (kt - 4 * B) * 128)
                    psc = ps.tile([128, 512], F32, tag="ps", name="psc")
                    nc.tensor.matmul(
                        psc[:, qoff:512],
                        kT[h][:, kt * 128:(kt + 1) * 128],
                        qT[h][:, B * 512 + qoff:(B + 1) * 512],
                        start=True, stop=True)
                    pt = work.tile([128, 512], BF16, tag="pt", bufs=4,
                                   name="pt")
                    nc.scalar.activation(pt[:, qoff:512], psc[:, qoff:512],
                                         AF.Exp, bias=nexpb_sb[:], scale=1.0)
                    if kt >= 4 * B:
                        # zero probs where query < key (incl. stale cols)
                        nc.gpsimd.affine_select(
                            out=pt[:], in_=pt[:], compare_op=ALU.is_ge,
                            fill=0.0, base=B * 512 - kt * 128,
                            channel_multiplier=-1, pattern=[[1, 512]])
                    pts[kt] = pt
                if step >= LAG:
                    kt = step - LAG
                    pt = pts.pop(kt)
                    qo = max(0, (kt - 4 * B) * 128)
                    nc.tensor.matmul(pa[h][:, qo:512],
                                     v_sb[:, kt, h * VD:(h + 1) * VD],
                                     pt[:, qo:512], start=(kt == 0),
                                     stop=(kt == nkt - 1))
                    nc.tensor.matmul(pone[h][:, qo:512], ones_bf[:],
                                     pt[:, qo:512], start=(kt == 0),
                                     stop=(kt == nkt - 1))
            serow[h] = stat.tile([1, 512], F32, tag="serow", name="serow")
            nc.vector.tensor_copy(serow[h][:], pone[h][0:1, :])
            if B == NSC - 1 and h == 0:
                # tail block: nothing follows to hide the epilogue chain, so
                # start h0's reciprocal chain under h1's kt loop
                epi0 = _norm_chain(serow[0], 0)
        # ---- normalization epilogue for both heads ----
        pcol = [None, None]
        rb4 = [None, None]
        prt = [None, None]
        if B == NSC - 1:
            pcol[0], rb4[0], prt[0] = epi0
            pcol[1], rb4[1], prt[1] = _norm_chain(serow[1], 1)
        else:
            for h in range(HPC):
                pcol[h] = ps.tile([128, 4], F32, tag="ps", name=f"pcol{h}")
                for qi in range(4):
                    nc.tensor.transpose(pcol[h][:, qi:qi + 1],
                                        serow[h][0:1, qi * 128:(qi + 1) * 128],
                                        one_f32[0:1, 0:1])
            for h in range(HPC):
                r4 = stat.tile([128, 4], F32, tag="r4", name="r4")
                nc.vector.reciprocal(r4[:], pcol[h][:])
                rb4[h] = stat.tile([128, 4], BF16, tag="rb4", name="rb4")
                nc.vector.tensor_copy(rb4[h][:], r4[:])
            for h in range(HPC):
                prt[h] = ps.tile([1, 512], BF16, tag="ps", name=f"prt{h}")
                for qi in range(4):
                    nc.tensor.transpose(prt[h][0:1, qi * 128:(qi + 1) * 128],
                                        rb4[h][:, qi:qi + 1], ident[:])
        for h in range(HPC):
            rrbf = stat.tile([1, 512], BF16, tag="rrbf", name="rrbf")
            nc.vector.tensor_copy(rrbf[:], prt[h][0:1, :])
            rbp = ps.tile([128, 512], F32, tag="ps", name="rbp")
            nc.tensor.matmul(rbp[:], ones_row[0:1, :], rrbf[0:1, :],
                             start=True, stop=True)
            rbc = work.tile([128, 512], BF16, tag="rbc")
            nc.scalar.copy(rbc[:], rbp[:])
            a = work.tile([128, 512], BF16, tag=f"at{h}", name=f"at{h}")
            nc.vector.tensor_tensor(a[:], pa[h][:], rbc[:], ALU.mult)
            at[h] = a
        return at

    def attn_oproj(B, at):
        for t in range(4):
            ot = outp.tile([128, 4, 512], BF16, tag="ot")
            for n in range(4):
                po = ps.tile([128, 512], F32, tag="ps", name="po")
                for h in range(HPC):
                    nc.tensor.matmul(po[:], at[h][:, t * 128:(t + 1) * 128],
                                     wo_sb[:, h, n * 512:(n + 1) * 512],
                                     start=(h == 0), stop=(h == HPC - 1))
                if n % 2 == 0:
                    nc.vector.tensor_copy(ot[:, n, :], po[:])
                else:
                    nc.scalar.copy(ot[:, n, :], po[:])
            nc.sync.dma_start(
                out=d["o"][(4 * B + t) * 128:(4 * B + t + 1) * 128, :],
                in_=ot[:])

    # =====================================================================
    # schedule: chunk c's rmsnorm-scale chain hides behind chunk c+1's
    # waves; attention superblock c-1 fills the region before them.
    stg_ = [None] * NSC
    sq_ = [None] * NSC

    proj_w1(0)
    stg_[0], sq_[0] = proj_w2(0)
    srow = stats_a(sq_[0])
    proj_w1(1)
    yt, yb = stats_b1(srow)
    stg_[1], sq_[1] = proj_w2(1)
    sbcb = stats_b2(yb)
    proj_mm2(0, stg_[0], sbcb, yt)

    for c in range(1, NSC):
        at = attn_core(c - 1)
        srow = stats_a(sq_[c])
        if c + 1 < NSC:
            proj_w1(c + 1)
            yt, yb = stats_b1(srow)
            stg_[c + 1], sq_[c + 1] = proj_w2(c + 1)
            sbcb = stats_b2(yb)
            proj_mm2(c, stg_[c], sbcb, yt)
            attn_oproj(c - 1, at)
        else:
            attn_oproj(c - 1, at)
            yt, yb = stats_b1(srow)
            sbcb = stats_b2(yb)
            proj_mm2(c, stg_[c], sbcb, yt)
    at = attn_core(NSC - 1)
    attn_oproj(NSC - 1, at)


# =========================================================================
# host side
# =========================================================================
_perm1 = np.concatenate([np.arange(0, ROPE, 2), np.arange(1, ROPE, 2)])


def _host_prep(inputs):
    hidden = np.ascontiguousarray(np.asarray(inputs["hidden_states"],
                                             dtype=np.float32)[0])
    cos = np.asarray(inputs["cos"], dtype=np.float32)[0]
    sin = np.asarray(inputs["sin"], dtype=np.float32)[0]
    w_q = np.asarray(inputs["w_q"], dtype=np.float32)
    w_kv_a = np.asarray(inputs["w_kv_a"], dtype=np.float32)
    ln_w = np.asarray(inputs["kv_a_ln_w"], dtype=np.float32)
    w_kv_b = np.asarray(inputs["w_kv_b"], dtype=np.float32)
    w_o = np.asarray(inputs["w_o"], dtype=np.float32)

    hT = np.ascontiguousarray(hidden.T)
    cosT = cos.T
    sinT = sin.T
    # cos rows duplicated: rows 0:64 for kpe (psum parts 0:64), 64:128 for q
    cosd = np.ascontiguousarray(np.concatenate([cosT, cosT], axis=0))
    # msin rows placed at the partitions of the x' operand they multiply
    msind = np.ascontiguousarray(np.concatenate(
        [sinT[32:64], -sinT[0:32], sinT[32:64], -sinT[0:32]], axis=0))

    kpe_cols = w_kv_a[:, KLR:]
    kpe_x = kpe_cols[:, _perm1]
    wkva_mod = np.ascontiguousarray(np.concatenate(
        [w_kv_a[:, :KLR], kpe_x, kpe_x], axis=1))            # [HID, 640]
    wkvb_all = w_kv_b * ln_w[:, None]

    bf = ml_dtypes.bfloat16
    in_maps = []
    for cid in range(NCORES):
        heads = [HPC * cid + i for i in range(HPC)]
        blocks = []
        for h in heads:
            wq_h = w_q[:, h * QD:(h + 1) * QD]
            blocks.append(np.concatenate(
                [wq_h[:, :NOPE], wq_h[:, NOPE:][:, _perm1]], axis=1))
        wq_mod = np.ascontiguousarray(np.concatenate(blocks, axis=1) * SCALE)

        nope_b = [wkvb_all[:, h * (NOPE + VD):h * (NOPE + VD) + NOPE]
                  for h in heads]
        v_b = [wkvb_all[:, h * (NOPE + VD) + NOPE:(h + 1) * (NOPE + VD)]
               for h in heads]
        wkvb_mod = np.ascontiguousarray(np.concatenate(nope_b + v_b, axis=1))

        wo_mod = np.ascontiguousarray(w_o[heads[0] * VD:(heads[-1] + 1) * VD, :])

        in_maps.append({"hT": hT.astype(bf), "wq": wq_mod.astype(bf),
                        "wkva": wkva_mod.astype(bf),
                        "wkvb": wkvb_mod.astype(bf), "wo": wo_mod.astype(bf),
                        "cosd": cosd.astype(bf), "msind": msind.astype(bf)})
    return in_maps


def _install_ntff_hook():
    """Make trace=True work under axon (antenv.axon_hooks is absent in this
    image; back it with trn_agent_boot's ctypes hook)."""
    try:
        import antenv
        if "antenv.axon_hooks" in sys.modules:
            return
        from trn_agent_boot.trn_boot import _ntff_profile_via_ctypes
        hook = _ntff_profile_via_ctypes("/opt/axon/libaxon_pjrt.so")
        mod = types.ModuleType("antenv.axon_hooks")
        mod.get_axon_ntff_profile_hook = lambda: hook
        mod.set_axon_ntff_profile_hook = lambda h: None
        sys.modules["antenv.axon_hooks"] = mod
        antenv.axon_hooks = mod
    except Exception:
        pass


_nc_cache = None
last_results = None


def kernel(**inputs):
    global _nc_cache, last_results
    _install_ntff_hook()
    if _nc_cache is None:
        _nc_cache = build_nc()
    in_maps = _host_prep(inputs)
    trace = bool(os.environ.get("BASS_TRACE"))
    res = bass_utils.run_bass_kernel_spmd(
        _nc_cache, in_maps, core_ids=list(range(NCORES)), trace=trace)
    last_results = res
    total = res.results[0]["o"].astype(np.float32)
    for c in range(1, NCORES):
        total = total + res.results[c]["o"]
    return total.reshape(1, S, HID)


# revision 23
# speedup vs baseline: 1.0218x; 1.0218x over previous
"""MLA (multi-latent attention) prefill kernel for Trainium2, 8 NeuronCores.

Tensor-parallel over heads: each of the 8 cores owns 2 of the 16 heads.
w_q / w_kv_b are column-sharded, w_o row-sharded; the small kv_a latent
projection is replicated. Per-core partial outputs are summed on the host
(the "all-reduce" of the o_proj).

v3 dataflow (per core, column-major [feature, seq] layouts):
  qT   = wq_mod.T  @ hT     [256, S]  (2 M-blocks: h0[nope|x'], h1[nope|x'])
  kvaT = wkva_mod.T @ hT    [640, S]  (latent 512, then k-x' twice --
  the duplicate keeps the stationary 128 wide, narrow matmuls measure ~35%
  slower per row)
  RoPE: x' = interleaved pe cols (folded into weights); the rotate-half
  operand y is built as signed partition-shifted DVE multiplies against
  partition-aligned +-sin tables; adds run on GpSimd.
  rmsnorm: latent staged to SBUF in bf16 (stg); squares on GpSimd; the
  partition+block reduction is a ones-matmul on the PE; rsqrt = quake
  bit-trick seed + 2 Newton steps on [128,4]-transposed columns (DVE ops
  ~170ns each, no ACT tables touched).  kv_b consumes the UNNORMALIZED
  bf16 latent, and the norm scale folds into the kT writes (broadcast row
  multiply) and v writes (per-partition tensor_scalar) -- so no PE matmul
  ever waits on the rsqrt chain.
  Attention is computed TRANSPOSED: scoresT[k, q] = kT_blk^T @ qT so the
  exp (ACT) writes probsT straight to SBUF -- no PE transposes and no
  PSUM->SBUF prob copies. The ACT engine only ever runs Exp and Copy
  (one activation-table load for the whole kernel).  Causal masking =
  gpsimd.affine_select zeroing invalid probs on diagonal key-blocks.
  Softmax sums come from a ones matmul accumulated in PSUM; 1/sum is
  computed on [128,4]-transposed columns and folded into the PSUM->SBUF
  copy of attnT.  o = attnT.T @ w_o chunks, both heads accumulated in
  PSUM, streamed out.

Matmul operands are bf16 (full-rate PE, fp32 PSUM accumulation); softmax
statistics and rmsnorm statistics stay fp32.
"""
import os
import sys
import types
import numpy as np
import ml_dtypes

import concourse.bass as bass
import concourse.mybir as mybir
import concourse.tile as tile
from concourse import bacc, bass_isa, bass_utils, masks

F32 = mybir.dt.float32
BF16 = mybir.dt.bfloat16
I32 = mybir.dt.int32

S, HID = 2048, 2048
H, NOPE, ROPE, VD, KLR = 16, 64, 64, 128, 512
QD = NOPE + ROPE          # 128
SCALE = QD ** -0.5
EPS = 1e-6
NCORES = 8
HPC = H // NCORES         # heads per core = 2

SC = 512                  # seq chunk for projections
NSC = S // SC             # 4
HC = HID // 128           # 16 hid chunks
EXPB = 20.0               # fixed exp bias (overflow headroom)


def build_nc():
    nc = bacc.Bacc("TRN2", target_bir_lowering=False, debug=False,
                   num_devices=NCORES)
    dr = {}
    dr["hT"] = nc.dram_tensor("hT", [HID, S], BF16, kind="ExternalInput")
    dr["wq"] = nc.dram_tensor("wq", [HID, 256], BF16, kind="ExternalInput")
    dr["wkva"] = nc.dram_tensor("wkva", [HID, 640], BF16, kind="ExternalInput")
    dr["wkvb"] = nc.dram_tensor("wkvb", [KLR, 384], BF16, kind="ExternalInput")
    dr["wo"] = nc.dram_tensor("wo", [HPC * VD, HID], BF16, kind="ExternalInput")
    dr["cosd"] = nc.dram_tensor("cosd", [128, S], BF16, kind="ExternalInput")
    dr["msind"] = nc.dram_tensor("msind", [128, S], BF16, kind="ExternalInput")
    dr["o"] = nc.dram_tensor("o", [S, HID], BF16, kind="ExternalOutput")

    with tile.TileContext(nc) as tc:
        build_tile_kernel(nc, tc, {k: v.ap() for k, v in dr.items()})
    nc.compile()
    return nc


def build_tile_kernel(nc, tc, d):
    from contextlib import ExitStack
    with ExitStack() as ctx:
        _build_tile_kernel(nc, tc, d, ctx)


def _build_tile_kernel(nc, tc, d, ctx):
    AF = mybir.ActivationFunctionType
    ALU = mybir.AluOpType

    consts = ctx.enter_context(tc.tile_pool(name="consts", bufs=1))
    big = ctx.enter_context(tc.tile_pool(name="big", bufs=1))
    work = ctx.enter_context(tc.tile_pool(name="work", bufs=2))
    stat = ctx.enter_context(tc.tile_pool(name="stat", bufs=2))
    outp = ctx.enter_context(tc.tile_pool(name="outp", bufs=2))
    ps = ctx.enter_context(tc.tile_pool(name="ps", bufs=8, space="PSUM"))

    # ---- input DMAs ------------------------------------------------------
    # full hT resident in SBUF; chunk-0 pieces land first
    hT_sb = consts.tile([128, HC, S], BF16)
    cos_sb = consts.tile([128, S], BF16)
    msin_sb = consts.tile([128, S], BF16)
    wkvb_sb = consts.tile([128, 4, 384], BF16)
    wo_sb = consts.tile([128, HPC, HID], BF16)
    for c in range(NSC):
        cs = slice(c * SC, (c + 1) * SC)
        for kp in range(HC // 2):
            nc.sync.dma_start(
                out=hT_sb[:, 2 * kp:2 * kp + 2, cs],
                in_=d["hT"][256 * kp:256 * (kp + 1), cs].rearrange(
                    "(k p) m -> p k m", p=128))
        if c == 0:
            nc.sync.dma_start(out=cos_sb[:], in_=d["cosd"])
            nc.sync.dma_start(out=msin_sb[:], in_=d["msind"])
            nc.sync.dma_start(out=wkvb_sb[:],
                              in_=d["wkvb"].rearrange("(k p) m -> p k m", p=128))
    wq_sb = consts.tile([128, HC, 256], BF16)
    wkva_sb = consts.tile([128, HC, 640], BF16)
    wq_r = d["wq"].rearrange("(k p) m -> p k m", p=128)
    wkva_r = d["wkva"].rearrange("(k p) m -> p k m", p=128)
    for k in range(HC):
        nc.scalar.dma_start(out=wq_sb[:, k, :], in_=wq_r[:, k, :])
        nc.scalar.dma_start(out=wkva_sb[:, k, :], in_=wkva_r[:, k, :])
    nc.sync.dma_start(out=wo_sb[:], in_=d["wo"].rearrange("(h p) n -> p h n", p=128))

    ones_bf = consts.tile([128, 128], BF16)
    nc.vector.memset(ones_bf[:], 1.0)
    one_f32 = consts.tile([1, 1], F32)
    nc.vector.memset(one_f32[:], 1.0)
    ident = consts.tile([128, 128], BF16)
    masks.make_identity(nc, ident[:])
    nexpb_sb = consts.tile([128, 1], F32)
    nc.vector.memset(nexpb_sb[:], -EXPB)
    ones_row = consts.tile([1, 128], BF16)
    nc.vector.memset(ones_row[:], 1.0)

    # ---- persistent activations -----------------------------------------
    qT = [big.tile([128, S], BF16, tag=f"qT{h}", name=f"qT{h}") for h in range(HPC)]
    kT = [big.tile([128, S], BF16, tag=f"kT{h}", name=f"kT{h}") for h in range(HPC)]
    v_sb = big.tile([128, S // 128, HPC * VD], BF16, tag="v")

    # =====================================================================
    def proj_w1(c):
        """q + shared k_pe wave and the rope epilogue."""
        cs = slice(c * SC, (c + 1) * SC)
        pq = [ps.tile([128, SC], F32, tag="ps", name=f"pq{i}") for i in range(HPC)]
        pkpe = ps.tile([128, SC], F32, tag="ps", name="pkpe")
        for k in range(HC):
            for h in range(HPC):
                nc.tensor.matmul(pq[h][:], wq_sb[:, k, h * 128:(h + 1) * 128],
                                 hT_sb[:, k, cs], start=(k == 0),
                                 stop=(k == HC - 1))
            nc.tensor.matmul(pkpe[:], wkva_sb[:, k, 512:640],
                             hT_sb[:, k, cs], start=(k == 0),
                             stop=(k == HC - 1))
        # rope: q' = x'*cos + y*sin, y = signed rotate-half of x'
        for h in range(HPC):
            nc.vector.tensor_copy(qT[h][0:64, cs], pq[h][0:64, :])
            t2 = work.tile([128, SC], F32, tag="t2", bufs=4)
            t3 = work.tile([128, SC], F32, tag="t2", bufs=4)
            nc.vector.tensor_tensor(t2[64:96, :], pq[h][96:128, :],
                                    msin_sb[96:128, cs], ALU.mult)
            nc.vector.tensor_tensor(t2[96:128, :], pq[h][64:96, :],
                                    msin_sb[64:96, cs], ALU.mult)
            nc.vector.tensor_tensor(t3[64:128, :], pq[h][64:128, :],
                                    cos_sb[64:128, cs], ALU.mult)
            nc.vector.tensor_tensor(qT[h][64:128, cs], t3[64:128, :],
                                    t2[64:128, :], ALU.add)
        tk = work.tile([128, SC], F32, tag="t2", bufs=4)
        tk3 = work.tile([128, SC], F32, tag="t2", bufs=4)
        nc.vector.tensor_tensor(tk[64:96, :], pkpe[32:64, :],
                                msin_sb[32:64, cs], ALU.mult)
        nc.vector.tensor_tensor(tk[96:128, :], pkpe[0:32, :],
                                msin_sb[0:32, cs], ALU.mult)
        nc.vector.tensor_tensor(tk3[64:128, :], pkpe[0:64, :],
                                cos_sb[0:64, cs], ALU.mult)
        nc.vector.tensor_tensor(kT[0][64:128, cs], tk3[64:128, :],
                                tk[64:128, :], ALU.add)
        nc.vector.tensor_copy(kT[1][64:128, cs], kT[0][64:128, cs])

    def proj_w2(c):
        """latent wave; stage to bf16 SBUF; squares on GpSimd."""
        cs = slice(c * SC, (c + 1) * SC)
        plat = [ps.tile([128, SC], F32, tag="ps", name=f"plat{i}") for i in range(4)]
        for k in range(HC):
            for m in range(4):
                nc.tensor.matmul(plat[m][:], wkva_sb[:, k, m * 128:(m + 1) * 128],
                                 hT_sb[:, k, cs], start=(k == 0),
                                 stop=(k == HC - 1))
        stg = work.tile([128, 4, SC], BF16, tag="stg", bufs=2)
        nc.vector.tensor_copy(stg[:, 0, :], plat[0][:])
        nc.vector.tensor_copy(stg[:, 1, :], plat[1][:])
        nc.scalar.copy(stg[:, 2, :], plat[2][:])
        nc.scalar.copy(stg[:, 3, :], plat[3][:])
        sq = work.tile([128, 4, SC], BF16, tag="sq", bufs=1)
        nc.scalar.activation(sq[:], stg[:], AF.Square)
        return stg, sq

    # rmsnorm scale chain, split so the PE pieces slot between waves and
    # no PE matmul ever waits on it (kv_b reads the unnormalized latent).
    def stats_a(sq):
        """PE reduction of sum-of-squares + row copy to SBUF."""
        pssq = ps.tile([128, SC], F32, tag="ps", name="pssq")
        for m in range(4):
            nc.tensor.matmul(pssq[:], ones_bf[:], sq[:, m, :],
                             start=(m == 0), stop=(m == 3))
        srow = stat.tile([1, SC], F32, tag="srow", name="srow")
        nc.vector.tensor_copy(srow[:], pssq[0:1, :])
        return srow

    def stats_b1(srow):
        """transpose sums to [128,4] columns; quake rsqrt on the columns."""
        pcol = ps.tile([128, 4], F32, tag="ps", name="pcolq")
        for qi in range(4):
            nc.tensor.transpose(pcol[:, qi:qi + 1],
                                srow[0:1, qi * 128:(qi + 1) * 128],
                                one_f32[0:1, 0:1])
        mt = stat.tile([128, 4], F32, tag="mt", name="mt")
        nc.vector.tensor_scalar(out=mt[:], in0=pcol[:], scalar1=1.0 / KLR,
                                scalar2=EPS, op0=ALU.mult, op1=ALU.add)
        ti = stat.tile([128, 4], I32, tag="ti", name="ti")
        nc.vector.tensor_scalar(out=ti[:], in0=mt.bitcast(I32)[:],
                                scalar1=1, scalar2=None,
                                op0=ALU.logical_shift_right)
        yt = stat.tile([128, 4], F32, tag="yt", name="yt")
        nc.vector.tensor_scalar(out=yt.bitcast(I32)[:], in0=ti[:],
                                scalar1=-1, scalar2=0x5F3759DF,
                                op0=ALU.mult, op1=ALU.add)
        y2 = stat.tile([128, 4], F32, tag="y2", name="y2")
        for _ in range(2):
            nc.vector.tensor_tensor(y2[:], yt[:], yt[:], ALU.mult)
            nc.vector.scalar_tensor_tensor(out=y2[:], in0=y2[:], scalar=-0.5,
                                           in1=mt[:], op0=ALU.mult,
                                           op1=ALU.mult)
            nc.vector.scalar_tensor_tensor(out=yt[:], in0=y2[:], scalar=1.5,
                                           in1=yt[:], op0=ALU.add,
                                           op1=ALU.mult)
        yb = stat.tile([128, 4], BF16, tag="yb", name="yb")
        nc.vector.tensor_copy(yb[:], yt[:])
        return yt, yb

    def stats_b2(yb):
        """columns back to a row; broadcast to all partitions."""
        prt = ps.tile([1, SC], BF16, tag="ps", name="prtq")
        for qi in range(4):
            nc.tensor.transpose(prt[0:1, qi * 128:(qi + 1) * 128],
                                yb[:, qi:qi + 1], ident[:])
        rrbf = stat.tile([1, SC], BF16, tag="rrbfq", name="rrbfq")
        nc.vector.tensor_copy(rrbf[:], prt[0:1, :])
        sbcp = ps.tile([128, SC], F32, tag="ps", name="sbcp")
        nc.tensor.matmul(sbcp[:], ones_row[0:1, :], rrbf[0:1, :],
                         start=True, stop=True)
        sbcb = work.tile([128, SC], BF16, tag="sbcb", bufs=2)
        nc.scalar.copy(sbcb[:], sbcp[:])
        return sbcb

    def proj_mm2(c, stg, sbcb, yt):
        """kv_b from the unnormalized latent; norm scale folded into the
        kT (row broadcast multiply) and v (per-partition scalar) writes."""
        cs = slice(c * SC, (c + 1) * SC)
        pnope = ps.tile([128, SC], F32, tag="ps", name="pnope")
        for kk in range(4):
            nc.tensor.matmul(pnope[:], wkvb_sb[:, kk, 0:128], stg[:, kk, :],
                             start=(kk == 0), stop=(kk == 3))
        nc.vector.tensor_tensor(kT[0][0:64, cs], pnope[0:64, :],
                                sbcb[0:64, :], ALU.mult)
        nc.vector.tensor_tensor(kT[1][0:64, cs], pnope[64:128, :],
                                sbcb[64:128, :], ALU.mult)
        for t in range(4):
            pv = ps.tile([128, HPC * VD], F32, tag="ps", name="pv")
            for kk in range(4):
                nc.tensor.matmul(pv[:], stg[:, kk, t * 128:(t + 1) * 128],
                                 wkvb_sb[:, kk, 128:384],
                                 start=(kk == 0), stop=(kk == 3))
            nc.vector.tensor_scalar_mul(v_sb[:, 4 * c + t, :], pv[:],
                                        yt[:, t:t + 1])

    # =====================================================================
    def _norm_chain(srow_h, h):
        """serow -> [128,4] columns -> reciprocal -> back to a [1,512] row."""
        pc = ps.tile([128, 4], F32, tag="ps", name=f"pcol{h}")
        for qi in range(4):
            nc.tensor.transpose(pc[:, qi:qi + 1],
                                srow_h[0:1, qi * 128:(qi + 1) * 128],
                                one_f32[0:1, 0:1])
        r4 = stat.tile([128, 4], F32, tag="r4", name="r4")
        nc.vector.reciprocal(r4[:], pc[:])
        rb = stat.tile([128, 4], BF16, tag="rb4", name="rb4")
        nc.vector.tensor_copy(rb[:], r4[:])
        pr = ps.tile([1, 512], BF16, tag="ps", name=f"prt{h}")
        for qi in range(4):
            nc.tensor.transpose(pr[0:1, qi * 128:(qi + 1) * 128],
                                rb[:, qi:qi + 1], ident[:])
        return pc, rb, pr

    def attn_core(B):
        """Transposed-scores attention for superblock B (512 queries), both
        heads.  Per key-block kt: scoresT (PE) -> exp (ACT, writes probsT to
        SBUF) -> [causal zero via affine_select on diagonal blocks (gpsimd)]
        -> attnT accumulate + ones sum accumulate (PE).  The 1/sumexp scale
        is folded into the PSUM->SBUF copy of attnT; its reciprocal chain
        (transpose->recip->transpose->broadcast) is scheduled after both
        heads so the PE never waits on it."""
        nkt = 4 * (B + 1)
        LAG = 2
        pa = [None, None]
        pone = [None, None]
        serow = [None, None]
        at = [None, None]
        for h in range(HPC):
            pa[h] = ps.tile([128, 512], F32, tag="ps", name=f"pa{h}")
            pone[h] = ps.tile([128, 512], F32, tag="ps", name=f"pone{h}")
            pts = {}
            for step in range(nkt + LAG):
                if step < nkt:
                    kt = step
                    qoff = max(0, (kt - 4 * B) * 128)
                    psc = ps.tile([128, 512], F32, tag="ps", name="psc")
                    nc.tensor.matmul(
                        psc[:, qoff:512],
                        kT[h][:, kt * 128:(kt + 1) * 128],
                        qT[h][:, B * 512 + qoff:(B + 1) * 512],
                        start=True, stop=True)
                    pt = work.tile([128, 512], BF16, tag="pt", bufs=4,
                                   name="pt")
                    nc.scalar.activation(pt[:, qoff:512], psc[:, qoff:512],
                                         AF.Exp, bias=nexpb_sb[:], scale=1.0)
                    if kt >= 4 * B:
                        # zero probs where query < key (incl. stale cols)
                        nc.gpsimd.affine_select(
                            out=pt[:], in_=pt[:], compare_op=ALU.is_ge,
                            fill=0.0, base=B * 512 - kt * 128,
                            channel_multiplier=-1, pattern=[[1, 512]])
                    pts[kt] = pt
                if step >= LAG:
                    kt = step - LAG
                    pt = pts.pop(kt)
                    qo = max(0, (kt - 4 * B) * 128)
                    nc.tensor.matmul(pa[h][:, qo:512],
                                     v_sb[:, kt, h * VD:(h + 1) * VD],
                                     pt[:, qo:512], start=(kt == 0),
                                     stop=(kt == nkt - 1))
                    nc.tensor.matmul(pone[h][:, qo:512], ones_bf[:],
                                     pt[:, qo:512], start=(kt == 0),
                                     stop=(kt == nkt - 1))
            serow[h] = stat.tile([1, 512], F32, tag="serow", name="serow")
            nc.vector.tensor_copy(serow[h][:], pone[h][0:1, :])
        # ---- normalization epilogue for both heads ----
        pcol = [None, None]
        for h in range(HPC):
            pcol[h] = ps.tile([128, 4], F32, tag="ps", name=f"pcol{h}")
            for qi in range(4):
                nc.tensor.transpose(pcol[h][:, qi:qi + 1],
                                    serow[h][0:1, qi * 128:(qi + 1) * 128],
                                    one_f32[0:1, 0:1])
        rb4 = [None, None]
        for h in range(HPC):
            r4 = stat.tile([128, 4], F32, tag="r4", name="r4")
            nc.vector.reciprocal(r4[:], pcol[h][:])
            rb4[h] = stat.tile([128, 4], BF16, tag="rb4", name="rb4")
            nc.vector.tensor_copy(rb4[h][:], r4[:])
        prt = [None, None]
        for h in range(HPC):
            prt[h] = ps.tile([1, 512], BF16, tag="ps", name=f"prt{h}")
            for qi in range(4):
                nc.tensor.transpose(prt[h][0:1, qi * 128:(qi + 1) * 128],
                                    rb4[h][:, qi:qi + 1], ident[:])
        for h in range(HPC):
            rrbf = stat.tile([1, 512], BF16, tag="rrbf", name="rrbf")
            nc.vector.tensor_copy(rrbf[:], prt[h][0:1, :])
            rbp = ps.tile([128, 512], F32, tag="ps", name="rbp")
            nc.tensor.matmul(rbp[:], ones_row[0:1, :], rrbf[0:1, :],
                             start=True, stop=True)
            rbc = work.tile([128, 512], BF16, tag="rbc")
            nc.scalar.copy(rbc[:], rbp[:])
            a = work.tile([128, 512], BF16, tag=f"at{h}", name=f"at{h}")
            nc.vector.tensor_tensor(a[:], pa[h][:], rbc[:], ALU.mult)
            at[h] = a
        return at

    def attn_oproj(B, at):
        for t in range(4):
            ot = outp.tile([128, 4, 512], BF16, tag="ot")
            for n in range(4):
                po = ps.tile([128, 512], F32, tag="ps", name="po")
                for h in range(HPC):
                    nc.tensor.matmul(po[:], at[h][:, t * 128:(t + 1) * 128],
                                     wo_sb[:, h, n * 512:(n + 1) * 512],
                                     start=(h == 0), stop=(h == HPC - 1))
                if n % 2 == 0:
                    nc.vector.tensor_copy(ot[:, n, :], po[:])
                else:
                    nc.scalar.copy(ot[:, n, :], po[:])
            nc.sync.dma_start(
                out=d["o"][(4 * B + t) * 128:(4 * B + t + 1) * 128, :],
                in_=ot[:])

    # =====================================================================
    # schedule: chunk c's rmsnorm-scale chain hides behind chunk c+1's
    # waves; attention superblock c-1 fills the region before them.
    stg_ = [None] * NSC
    sq_ = [None] * NSC

    proj_w1(0)
    stg_[0], sq_[0] = proj_w2(0)
    srow = stats_a(sq_[0])
    proj_w1(1)
    yt, yb = stats_b1(srow)
    stg_[1], sq_[1] = proj_w2(1)
    sbcb = stats_b2(yb)
    proj_mm2(0, stg_[0], sbcb, yt)

    for c in range(1, NSC):
        at = attn_core(c - 1)
        srow = stats_a(sq_[c])
        if c + 1 < NSC:
            proj_w1(c + 1)
            yt, yb = stats_b1(srow)
            stg_[c + 1], sq_[c + 1] = proj_w2(c + 1)
            sbcb = stats_b2(yb)
            proj_mm2(c, stg_[c], sbcb, yt)
            attn_oproj(c - 1, at)
        else:
            attn_oproj(c - 1, at)
            yt, yb = stats_b1(srow)
            sbcb = stats_b2(yb)
            proj_mm2(c, stg_[c], sbcb, yt)
    at = attn_core(NSC - 1)
    attn_oproj(NSC - 1, at)


# =========================================================================
# host side
# =========================================================================
_perm1 = np.concatenate([np.arange(0, ROPE, 2), np.arange(1, ROPE, 2)])


def _host_prep(inputs):
    hidden = np.ascontiguousarray(np.asarray(inputs["hidden_states"],
                                             dtype=np.float32)[0])
    cos = np.asarray(inputs["cos"], dtype=np.float32)[0]
    sin = np.asarray(inputs["sin"], dtype=np.float32)[0]
    w_q = np.asarray(inputs["w_q"], dtype=np.float32)
    w_kv_a = np.asarray(inputs["w_kv_a"], dtype=np.float32)
    ln_w = np.asarray(inputs["kv_a_ln_w"], dtype=np.float32)
    w_kv_b = np.asarray(inputs["w_kv_b"], dtype=np.float32)
    w_o = np.asarray(inputs["w_o"], dtype=np.float32)

    hT = np.ascontiguousarray(hidden.T)
    cosT = cos.T
    sinT = sin.T
    # cos rows duplicated: rows 0:64 for kpe (psum parts 0:64), 64:128 for q
    cosd = np.ascontiguousarray(np.concatenate([cosT, cosT], axis=0))
    # msin rows placed at the partitions of the x' operand they multiply
    msind = np.ascontiguousarray(np.concatenate(
        [sinT[32:64], -sinT[0:32], sinT[32:64], -sinT[0:32]], axis=0))

    kpe_cols = w_kv_a[:, KLR:]
    kpe_x = kpe_cols[:, _perm1]
    wkva_mod = np.ascontiguousarray(np.concatenate(
        [w_kv_a[:, :KLR], kpe_x, kpe_x], axis=1))            # [HID, 640]
    wkvb_all = w_kv_b * ln_w[:, None]

    bf = ml_dtypes.bfloat16
    in_maps = []
    for cid in range(NCORES):
        heads = [HPC * cid + i for i in range(HPC)]
        blocks = []
        for h in heads:
            wq_h = w_q[:, h * QD:(h + 1) * QD]
            blocks.append(np.concatenate(
                [wq_h[:, :NOPE], wq_h[:, NOPE:][:, _perm1]], axis=1))
        wq_mod = np.ascontiguousarray(np.concatenate(blocks, axis=1) * SCALE)

        nope_b = [wkvb_all[:, h * (NOPE + VD):h * (NOPE + VD) + NOPE]
                  for h in heads]
        v_b = [wkvb_all[:, h * (NOPE + VD) + NOPE:(h + 1) * (NOPE + VD)]
               for h in heads]
        wkvb_mod = np.ascontiguousarray(np.concatenate(nope_b + v_b, axis=1))

        wo_mod = np.ascontiguousarray(w_o[heads[0] * VD:(heads[-1] + 1) * VD, :])

        in_maps.append({"hT": hT.astype(bf), "wq": wq_mod.astype(bf),
                        "wkva": wkva_mod.astype(bf),
                        "wkvb": wkvb_mod.astype(bf), "wo": wo_mod.astype(bf),
                        "cosd": cosd.astype(bf), "msind": msind.astype(bf)})
    return in_maps


def _install_ntff_hook():
    """Make trace=True work under axon (antenv.axon_hooks is absent in this
    image; back it with trn_agent_boot's ctypes hook)."""
    try:
        import antenv
        if "antenv.axon_hooks" in sys.modules:
            return
        from trn_agent_boot.trn_boot import _ntff_profile_via_ctypes
        hook = _ntff_profile_via_ctypes("/opt/axon/libaxon_pjrt.so")
        mod = types.ModuleType("antenv.axon_hooks")
        mod.get_axon_ntff_profile_hook = lambda: hook
        mod.set_axon_ntff_profile_hook = lambda h: None
        sys.modules["antenv.axon_hooks"] = mod
        antenv.axon_hooks = mod
    except Exception:
        pass


_nc_cache = None
last_results = None


def kernel(**inputs):
    global _nc_cache, last_results
    _install_ntff_hook()
    if _nc_cache is None:
        _nc_cache = build_nc()
    in_maps = _host_prep(inputs)
    trace = bool(os.environ.get("BASS_TRACE"))
    res = bass_utils.run_bass_kernel_spmd(
        _nc_cache, in_maps, core_ids=list(range(NCORES)), trace=trace)
    last_results = res
    total = res.results[0]["o"].astype(np.float32)
    for c in range(1, NCORES):
        total = total + res.results[c]["o"]
    return total.reshape(1, S, HID)
